# revision 5
# baseline (speedup 1.0000x reference)
"""Trainium2 Bass kernel for nn_MLP_4337916970028.

Computes: out = gelu(x @ up) @ down^T where
  up   = spmm(S, fwht(sign * w_up, 1/sqrt(N)).T)        [1024, 4096]
  down = spmm(S, fwht(sign * w_down.T, 1/sqrt(N)).T)    [1024, 4096]
with S the [1024, 8192] one-nonzero-per-column JL projection.

Algebra used on device: up = P @ w_up^T, down = P @ w_down, where
P = scale * S_dense @ H_8192 * diag(sign).  P is computed per-core
(128-row slice) as: 6 VectorE butterfly stages (H64 factor, free-axis
distances 128..4096) -> PE transpose -> H128 factor as TensorE matmuls
-> sign*scale on eviction.  P^T slices are AllGathered; the dense
P @ w projections run in float32r (full PE rate, ~1e-4); main matmuls
(x @ up, h @ down^T) also run in float32r with exact-Gelu on ScalarE.

Sharding: preprocessing is sharded over the 4096 hidden dim (512/core),
main phase is data-parallel over tokens (2048/core).
"""
import math
import os
import sys
import types

sys.path.insert(0, "/opt/trn_rl_repo")
import numpy as np  # noqa: E402

import concourse.bass as bass  # noqa: E402
import concourse.mybir as mybir  # noqa: E402
import concourse.tile as tile  # noqa: E402
from concourse import bacc  # noqa: E402
from concourse.bass_utils import run_bass_kernel_spmd  # noqa: E402
from concourse.masks import make_identity  # noqa: E402

F32 = mybir.dt.float32
F32R = mybir.dt.float32r
AF = mybir.ActivationFunctionType

NC = 8
R = 1024      # n_embd
C = 8192      # hadamard dim N
D = 4096      # hidden 4*n_embd
T = 16384     # tokens
DS = D // NC  # 512 hidden per core (preproc shard)
TS = T // NC  # 2048 tokens per core (main shard)
SCALE = 1.0 / math.sqrt(C)

_NC_CACHE = None
last_exec_time_ns = None


def _register_ntff_hook():
    try:
        import antenv.axon_hooks  # noqa: F401
        return
    except ImportError:
        pass
    try:
        from trn_agent_boot.trn_boot import _ntff_profile_via_ctypes
        hook = _ntff_profile_via_ctypes("/opt/axon/libaxon_pjrt.so")
    except Exception:
        return
    mod = types.ModuleType("antenv.axon_hooks")
    mod._hook = hook
    mod.get_axon_ntff_profile_hook = lambda: mod._hook
    mod.set_axon_ntff_profile_hook = lambda h: setattr(mod, "_hook", h)
    sys.modules["antenv.axon_hooks"] = mod
    import antenv
    antenv.axon_hooks = mod


def _hadamard(n):
    H = np.array([[1.0]], dtype=np.float64)
    while H.shape[0] < n:
        H = np.block([[H, H], [H, -H]])
    return H


def _build():
    nc = bacc.Bacc("TRN2", target_bir_lowering=False, debug=False, num_devices=NC)
    s_in = nc.dram_tensor("s_in", [128, C], F32, kind="ExternalInput").ap()
    sign_in = nc.dram_tensor("sign_in", [128, 64], F32, kind="ExternalInput").ap()
    h128_in = nc.dram_tensor("h128_in", [128, 128], F32R, kind="ExternalInput").ap()
    wupt_in = nc.dram_tensor("wupt_in", [C, DS], F32R, kind="ExternalInput").ap()
    wdn_in = nc.dram_tensor("wdn_in", [C, DS], F32R, kind="ExternalInput").ap()
    xt_in = nc.dram_tensor("xt_in", [R, TS], F32R, kind="ExternalInput").ap()
    out_ext = nc.dram_tensor("out", [TS, R], F32, kind="ExternalOutput").ap()

    # slot order: even i's first, then odd (parity classes of the H64 fwht)
    order = list(range(0, 64, 2)) + list(range(1, 64, 2))

    with tile.TileContext(nc) as tc:
        with tc.tile_pool(name="dram", bufs=1, space="DRAM") as dram:
            pt_loc = [dram.tile([128, C // 2], F32, name=f"pt_loc{o}") for o in range(2)]
            pt_all = [dram.tile([NC * 128, C // 2], F32, addr_space="Shared",
                                name=f"pt_all{o}") for o in range(2)]
            up_loc = dram.tile([R, DS], F32)
            up_all = dram.tile([NC * R, DS], F32, addr_space="Shared")
            dn_loc = dram.tile([DS, R], F32)
            dn_all = dram.tile([D, R], F32, addr_space="Shared")
            h_dram = dram.tile([D, TS], F32)

            # ================= Phase A: P^T slice =================
            with (
                tc.tile_pool(name="pre", bufs=1) as pre,
                tc.tile_pool(name="ps_a", bufs=3, space="PSUM") as ps_a,
            ):
                s0 = pre.tile([128, C], F32)
                s1 = pre.tile([128, C], F32)
                nc.sync.dma_start(s0[:], s_in[:])
                sign_sc = pre.tile([128, 64], F32)
                nc.sync.dma_start(sign_sc[:], sign_in[:])
                nc.vector.tensor_scalar_mul(sign_sc[:], sign_sc[:], SCALE)
                ident = pre.tile([128, 128], F32)
                make_identity(nc, ident[:])
                h128 = pre.tile([128, 128], F32R)
                nc.sync.dma_start(h128[:], h128_in[:])

                # stage 0: butterfly distance 128 over full tensor (s0 -> s1)
                a = s0[:].rearrange("p (nb two h) -> p nb two h", two=2, h=128)
                y = s1[:].rearrange("p (nb two h) -> p nb two h", two=2, h=128)
                nc.vector.tensor_add(y[:, :, 0, :], a[:, :, 0, :], a[:, :, 1, :])
                nc.vector.tensor_sub(y[:, :, 1, :], a[:, :, 0, :], a[:, :, 1, :])

                s1t = pre.tile([128, C], F32R)   # [j', (slot, rl)]
                ptt = pre.tile([128, C], F32R)   # [j, (slot, rl)]

                for par in range(2):
                    # stages 1..5 restricted to parity class `par` (s1 -> s0,
                    # ping-pong; odd number of steps ends in s0)
                    cur, nxt = s1, s0
                    for st in range(1, 6):
                        hh = 2 ** (st - 1)
                        nb = 64 // (2 * hh * 2)
                        a = cur[:].rearrange(
                            "p (nb two hh par j) -> p nb two hh par j",
                            two=2, hh=hh, par=2, j=128)
                        y = nxt[:].rearrange(
                            "p (nb two hh par j) -> p nb two hh par j",
                            two=2, hh=hh, par=2, j=128)
                        nc.vector.tensor_add(y[:, :, 0, :, par, :],
                                             a[:, :, 0, :, par, :], a[:, :, 1, :, par, :])
                        nc.vector.tensor_sub(y[:, :, 1, :, par, :],
                                             a[:, :, 0, :, par, :], a[:, :, 1, :, par, :])
                        cur, nxt = nxt, cur
                    # class columns now final in s0; process its 32 i-tiles
                    for sg in range(8):  # groups of 4 slots
                        for u in range(4):
                            slot = 32 * par + 4 * sg + u
                            i = order[slot]
                            tp = ps_a.tile([128, 128], F32, tag="tp")
                            nc.tensor.transpose(tp[:], s0[:, 128 * i:128 * (i + 1)],
                                                ident[:])
                            nc.scalar.activation(
                                s1t[:, 128 * slot:128 * (slot + 1)], tp[:], AF.Copy)
                        sl0 = 32 * par + 4 * sg
                        pp = ps_a.tile([128, 512], F32, tag="pp")
                        nc.tensor.matmul(pp[:], h128[:],
                                         s1t[:, 512 * (sl0 // 4):512 * (sl0 // 4 + 1)],
                                         start=True, stop=True)
                        for u in range(4):
                            slot = sl0 + u
                            i = order[slot]
                            nc.scalar.activation(
                                ptt[:, 128 * slot:128 * (slot + 1)],
                                pp[:, 128 * u:128 * (u + 1)],
                                AF.Copy, scale=sign_sc[:, i:i + 1])
                    nc.sync.dma_start(
                        pt_loc[par][:],
                        ptt[:, 4096 * par:4096 * (par + 1)].bitcast(F32))

            for par in range(2):
                nc.gpsimd.collective_compute(
                    "AllGather", mybir.AluOpType.bypass,
                    replica_groups=[list(range(NC))],
                    ins=[pt_loc[par].opt()], outs=[pt_all[par].opt()])

            # ============ Phase B: up, then down (gathers hide downstream) ====
            def proj_pass(w_in, out_sl_dtype, pool_sfx):
                """PSUM-resident pass over slots: out[128m+rl, d'] for all m."""
                with (
                    tc.tile_pool(name=f"pb{pool_sfx}", bufs=8) as pb,
                    tc.tile_pool(name=f"pbw{pool_sfx}", bufs=16) as pbw,
                    tc.tile_pool(name=f"pbo{pool_sfx}", bufs=1) as pbo,
                    tc.tile_pool(name=f"ps_b{pool_sfx}", bufs=1, space="PSUM") as ps_b,
                ):
                    psu = [ps_b.tile([128, DS], F32, tag=f"psu{m}", name=f"psu{m}")
                           for m in range(NC)]
                    for slot in range(64):
                        par, il = slot // 32, slot % 32
                        i = order[slot]
                        pti = pb.tile([128, NC * 128], F32R, tag="pti")
                        nc.sync.dma_start(
                            pti[:].rearrange("p (m t) -> p m t", m=NC),
                            pt_all[par][:, 128 * il:128 * (il + 1)].bitcast(F32R)
                            .rearrange("(m p) t -> p m t", p=128))
                        wi = pbw.tile([128, DS], F32R, tag="wi")
                        nc.sync.dma_start(wi[:], w_in[128 * i:128 * (i + 1), :])
                        for m in range(NC):
                            nc.tensor.matmul(psu[m][:],
                                             pti[:, 128 * m:128 * (m + 1)], wi[:],
                                             start=(slot == 0), stop=(slot == 63))
                    sl = pbo.tile([128, NC * DS], out_sl_dtype, name=f"sl{pool_sfx}")
                    for m in range(NC):
                        nc.scalar.activation(sl[:, DS * m:DS * (m + 1)], psu[m][:],
                                             AF.Copy)
                    return sl

            up_sl = proj_pass(wupt_in, F32R, "u")
            nc.sync.dma_start(
                up_loc.rearrange("(m p) d -> p m d", p=128),
                up_sl[:].bitcast(F32).rearrange("p (m d) -> p m d", m=NC))
            nc.gpsimd.collective_compute(
                "AllGather", mybir.AluOpType.bypass,
                replica_groups=[list(range(NC))],
                ins=[up_loc.opt()], outs=[up_all.opt()])

            dn_sl = proj_pass(wdn_in, F32, "d")
            with (
                tc.tile_pool(name="dnt", bufs=1) as dnt_pool,
                tc.tile_pool(name="ps_t", bufs=3, space="PSUM") as ps_t,
            ):
                ident2 = dnt_pool.tile([128, 128], F32)
                make_identity(nc, ident2[:])
                dnt = dnt_pool.tile([128, 4 * R], F32R)  # [dp, (a, r)]
                for a in range(4):
                    for m in range(NC):
                        tp = ps_t.tile([128, 128], F32, tag="tp2")
                        nc.tensor.transpose(
                            tp[:], dn_sl[:, DS * m + 128 * a:DS * m + 128 * (a + 1)],
                            ident2[:])
                        nc.scalar.activation(
                            dnt[:, R * a + 128 * m:R * a + 128 * (m + 1)], tp[:],
                            AF.Copy)
                nc.sync.dma_start(
                    dn_loc.rearrange("(a p) r -> p a r", p=128),
                    dnt[:].bitcast(F32).rearrange("p (a r) -> p a r", a=4))
            nc.gpsimd.collective_compute(
                "AllGather", mybir.AluOpType.bypass,
                replica_groups=[list(range(NC))],
                ins=[dn_loc.opt()], outs=[dn_all.opt()])

            # ================= Phase C: main matmuls =================
            with tc.tile_pool(name="cdn", bufs=1) as cdn:
                dn_a = cdn.tile([128, 20 * R], F32R)  # [p, (dk<20, r)]
                nc.sync.dma_start(
                    dn_a[:].rearrange("p (dk r) -> p dk r", dk=20),
                    dn_all[0:20 * 128, :].bitcast(F32R)
                    .rearrange("(dk p) r -> p dk r", p=128))

                # mm1: h^T = (x @ up)^T with gelu, streamed to DRAM
                with (
                    tc.tile_pool(name="c1", bufs=1) as c1,
                    tc.tile_pool(name="c1s", bufs=3) as c1s,
                    tc.tile_pool(name="ps_c1", bufs=4, space="PSUM") as ps_c1,
                ):
                    xt_sb = c1.tile([128, NC * TS], F32R)  # [p, (rk, t)]
                    nc.sync.dma_start(
                        xt_sb[:].rearrange("p (rk t) -> p rk t", rk=NC),
                        xt_in.rearrange("(rk p) t -> p rk t", p=128))
                    for g in range(NC):
                        upg = c1s.tile([128, NC * DS], F32R, tag="upg")  # [p,(rk,d')]
                        nc.sync.dma_start(
                            upg[:].rearrange("p (rk d) -> p rk d", rk=NC),
                            up_all[R * g:R * (g + 1), :].bitcast(F32R)
                            .rearrange("(rk p) d -> p rk d", p=128))
                        for dtg in range(4):
                            for tq in range(4):
                                ph = ps_c1.tile([128, 512], F32, tag="ph")
                                for rk in range(NC):
                                    nc.tensor.matmul(
                                        ph[:],
                                        upg[:, DS * rk + 128 * dtg:DS * rk + 128 * (dtg + 1)],
                                        xt_sb[:, TS * rk + 512 * tq:TS * rk + 512 * (tq + 1)],
                                        start=(rk == 0), stop=(rk == NC - 1))
                                ht = c1s.tile([128, 512], F32R, tag="ht", bufs=4)
                                nc.scalar.activation(ht[:], ph[:], AF.Gelu)
                                d0 = DS * g + 128 * dtg
                                nc.sync.dma_start(
                                    h_dram[d0:d0 + 128, 512 * tq:512 * (tq + 1)]
                                    .bitcast(F32R),
                                    ht[:])

                # mm2: out = h @ down^T
                with (
                    tc.tile_pool(name="c2", bufs=1) as c2,
                    tc.tile_pool(name="c2s", bufs=4) as c2s,
                    tc.tile_pool(name="ps_c2", bufs=4, space="PSUM") as ps_c2,
                ):
                    dn_b = c2.tile([128, 12 * R], F32R)  # [p, (dk>=20, r)]
                    nc.sync.dma_start(
                        dn_b[:].rearrange("p (dk r) -> p dk r", dk=12),
                        dn_all[20 * 128:D, :].bitcast(F32R)
                        .rearrange("(dk p) r -> p dk r", p=128))
                    for tt in range(16):
                        hcol = c2s.tile([128, 32 * 128], F32R, tag="hcol")
                        nc.sync.dma_start(
                            hcol[:].rearrange("p (dk t) -> p dk t", dk=32),
                            h_dram[:, 128 * tt:128 * (tt + 1)].bitcast(F32R)
                            .rearrange("(dk p) t -> p dk t", p=128))
                        for rh in range(2):
                            po = ps_c2.tile([128, 512], F32, tag="po")
                            for dk in range(32):
                                src, dkl = (dn_a, dk) if dk < 20 else (dn_b, dk - 20)
                                nc.tensor.matmul(
                                    po[:], hcol[:, 128 * dk:128 * (dk + 1)],
                                    src[:, R * dkl + 512 * rh:R * dkl + 512 * (rh + 1)],
                                    start=(dk == 0), stop=(dk == 31))
                            ot = c2s.tile([128, 512], F32, tag="ot", bufs=4)
                            nc.vector.tensor_copy(ot[:], po[:])
                            nc.sync.dma_start(
                                out_ext[128 * tt:128 * (tt + 1), 512 * rh:512 * (rh + 1)],
                                ot[:])

    nc.compile()
    return nc


def _get_nc():
    global _NC_CACHE
    if _NC_CACHE is None:
        _NC_CACHE = _build()
    return _NC_CACHE


def kernel(x, random_sign, proj_indices, proj_values, w_up, w_down):
    global last_exec_time_ns
    x = np.ascontiguousarray(np.asarray(x, dtype=np.float32))
    sign = np.asarray(random_sign, dtype=np.float32)
    pi = np.asarray(proj_indices)
    pv = np.asarray(proj_values, dtype=np.float32)
    w_up = np.asarray(w_up, dtype=np.float32)
    w_down = np.asarray(w_down, dtype=np.float32)

    # ---- host marshalling ----
    S = np.zeros((R, C), dtype=np.float32)
    np.add.at(S, (pi[0].astype(np.int64), pi[1].astype(np.int64)), pv)
    sign_host = np.ascontiguousarray(sign.reshape(64, 128).T)
    h128 = np.ascontiguousarray(_hadamard(128).astype(np.float32))
    xT = np.ascontiguousarray(x.T)
    wupT = np.ascontiguousarray(w_up.T)

    in_maps = []
    for k in range(NC):
        in_maps.append({
            "s_in": np.ascontiguousarray(S[128 * k:128 * (k + 1), :]),
            "sign_in": sign_host,
            "h128_in": h128,
            "wupt_in": np.ascontiguousarray(wupT[:, DS * k:DS * (k + 1)]),
            "wdn_in": np.ascontiguousarray(w_down[:, DS * k:DS * (k + 1)]),
            "xt_in": np.ascontiguousarray(xT[:, TS * k:TS * (k + 1)]),
        })

    trace = bool(os.environ.get("KERNEL_TRACE"))
    if trace:
        _register_ntff_hook()
    nc = _get_nc()
    res = run_bass_kernel_spmd(nc, in_maps, core_ids=list(range(NC)), trace=trace)
    last_exec_time_ns = res.exec_time_ns
    return np.concatenate([res.results[k]["out"] for k in range(NC)], axis=0)


# revision 6
# speedup vs baseline: 1.0062x; 1.0062x over previous
"""Trainium2 Bass kernel for nn_MLP_4337916970028.

Computes: out = gelu(x @ up) @ down^T where
  up   = spmm(S, fwht(sign * w_up, 1/sqrt(N)).T)        [1024, 4096]
  down = spmm(S, fwht(sign * w_down.T, 1/sqrt(N)).T)    [1024, 4096]
with S the [1024, 8192] one-nonzero-per-column JL projection.

Algebra used on device: up = P @ w_up^T, down = P @ w_down, where
P = scale * S_dense @ H_8192 * diag(sign).  P is computed per-core
(128-row slice) as: 6 VectorE butterfly stages (H64 factor, free-axis
distances 128..4096) -> PE transpose -> H128 factor as TensorE matmuls
-> sign*scale on eviction.  P^T slices are AllGathered; the dense
P @ w projections run in float32r (full PE rate, ~1e-4); main matmuls
(x @ up, h @ down^T) also run in float32r with exact-Gelu on ScalarE.

Sharding: preprocessing is sharded over the 4096 hidden dim (512/core),
main phase is data-parallel over tokens (2048/core).
"""
import math
import os
import sys
import types

sys.path.insert(0, "/opt/trn_rl_repo")
import numpy as np  # noqa: E402

import concourse.bass as bass  # noqa: E402
import concourse.mybir as mybir  # noqa: E402
import concourse.tile as tile  # noqa: E402
from concourse import bacc  # noqa: E402
from concourse.bass_utils import run_bass_kernel_spmd  # noqa: E402
from concourse.masks import make_identity  # noqa: E402

F32 = mybir.dt.float32
F32R = mybir.dt.float32r
AF = mybir.ActivationFunctionType

NC = 8
R = 1024      # n_embd
C = 8192      # hadamard dim N
D = 4096      # hidden 4*n_embd
T = 16384     # tokens
DS = D // NC  # 512 hidden per core (preproc shard)
TS = T // NC  # 2048 tokens per core (main shard)
SCALE = 1.0 / math.sqrt(C)

_NC_CACHE = None
last_exec_time_ns = None


def _register_ntff_hook():
    try:
        import antenv.axon_hooks  # noqa: F401
        return
    except ImportError:
        pass
    try:
        from trn_agent_boot.trn_boot import _ntff_profile_via_ctypes
        hook = _ntff_profile_via_ctypes("/opt/axon/libaxon_pjrt.so")
    except Exception:
        return
    mod = types.ModuleType("antenv.axon_hooks")
    mod._hook = hook
    mod.get_axon_ntff_profile_hook = lambda: mod._hook
    mod.set_axon_ntff_profile_hook = lambda h: setattr(mod, "_hook", h)
    sys.modules["antenv.axon_hooks"] = mod
    import antenv
    antenv.axon_hooks = mod


def _hadamard(n):
    H = np.array([[1.0]], dtype=np.float64)
    while H.shape[0] < n:
        H = np.block([[H, H], [H, -H]])
    return H


def _build():
    nc = bacc.Bacc("TRN2", target_bir_lowering=False, debug=False, num_devices=NC)
    s_in = nc.dram_tensor("s_in", [128, C], F32, kind="ExternalInput").ap()
    sign_in = nc.dram_tensor("sign_in", [128, 64], F32, kind="ExternalInput").ap()
    h128_in = nc.dram_tensor("h128_in", [128, 128], F32R, kind="ExternalInput").ap()
    wupt_in = nc.dram_tensor("wupt_in", [C, DS], F32R, kind="ExternalInput").ap()
    wdn_in = nc.dram_tensor("wdn_in", [C, DS], F32R, kind="ExternalInput").ap()
    xt_in = nc.dram_tensor("xt_in", [R, TS], F32R, kind="ExternalInput").ap()
    out_ext = nc.dram_tensor("out", [TS, R], F32, kind="ExternalOutput").ap()

    # slot order: even i's first, then odd (parity classes of the H64 fwht)
    order = list(range(0, 64, 2)) + list(range(1, 64, 2))

    with tile.TileContext(nc) as tc:
        with tc.tile_pool(name="dram", bufs=1, space="DRAM") as dram:
            pt_loc = [dram.tile([128, 2048], F32, name=f"pt_loc{o}") for o in range(4)]
            pt_all = [dram.tile([NC * 128, 2048], F32, addr_space="Shared",
                                name=f"pt_all{o}") for o in range(4)]
            up_loc = dram.tile([R, DS], F32)
            up_all = dram.tile([NC * R, DS], F32, addr_space="Shared")
            dn_loc = dram.tile([DS, R], F32)
            dn_all = dram.tile([D, R], F32, addr_space="Shared")
            h_dram = dram.tile([D, TS], F32)

            with tc.tile_pool(name="warm", bufs=1) as warm:
                # x^T prefetched immediately; lives through A, B, mm1
                xt_sb = warm.tile([128, NC * TS], F32R)  # [p, (rk, t)]
                nc.sync.dma_start(
                    xt_sb[:].rearrange("p (rk t) -> p rk t", rk=NC),
                    xt_in.rearrange("(rk p) t -> p rk t", p=128))

                # ================= Phase A: P^T slice =================
                with (
                    tc.tile_pool(name="pre", bufs=1) as pre,
                    tc.tile_pool(name="pres", bufs=3) as pres,
                    tc.tile_pool(name="ps_a", bufs=3, space="PSUM") as ps_a,
                ):
                    s0 = pre.tile([128, C], F32)
                    s1 = pre.tile([128, C], F32)
                    nc.sync.dma_start(s0[:], s_in[:])
                    sign_sc = pre.tile([128, 64], F32)
                    nc.sync.dma_start(sign_sc[:], sign_in[:])
                    nc.vector.tensor_scalar_mul(sign_sc[:], sign_sc[:], SCALE)
                    ident = pre.tile([128, 128], F32)
                    make_identity(nc, ident[:])
                    h128 = pre.tile([128, 128], F32R)
                    nc.sync.dma_start(h128[:], h128_in[:])

                    # stage 0: butterfly distance 128 over full tensor (s0 -> s1)
                    a = s0[:].rearrange("p (nb two h) -> p nb two h", two=2, h=128)
                    y = s1[:].rearrange("p (nb two h) -> p nb two h", two=2, h=128)
                    nc.vector.tensor_add(y[:, :, 0, :], a[:, :, 0, :], a[:, :, 1, :])
                    nc.vector.tensor_sub(y[:, :, 1, :], a[:, :, 0, :], a[:, :, 1, :])

                    for par in range(2):
                        # stages 1..5 on parity class `par` (ping-pong s1->s0)
                        cur, nxt = s1, s0
                        for st in range(1, 6):
                            hh = 2 ** (st - 1)
                            a = cur[:].rearrange(
                                "p (nb two hh par j) -> p nb two hh par j",
                                two=2, hh=hh, par=2, j=128)
                            y = nxt[:].rearrange(
                                "p (nb two hh par j) -> p nb two hh par j",
                                two=2, hh=hh, par=2, j=128)
                            nc.vector.tensor_add(
                                y[:, :, 0, :, par, :],
                                a[:, :, 0, :, par, :], a[:, :, 1, :, par, :])
                            nc.vector.tensor_sub(
                                y[:, :, 1, :, par, :],
                                a[:, :, 0, :, par, :], a[:, :, 1, :, par, :])
                            cur, nxt = nxt, cur
                        # class columns final in s0; emit two 8MB pt chunks
                        for half in range(2):
                            ch = 2 * par + half
                            pttc = pres.tile([128, 2048], F32R, tag="pttc", bufs=2)
                            for sg in range(4):
                                s1tg = pres.tile([128, 512], F32R, tag="s1tg")
                                for u in range(4):
                                    slot = 16 * ch + 4 * sg + u
                                    i = order[slot]
                                    tp = ps_a.tile([128, 128], F32, tag="tp")
                                    nc.tensor.transpose(
                                        tp[:], s0[:, 128 * i:128 * (i + 1)], ident[:])
                                    nc.scalar.activation(
                                        s1tg[:, 128 * u:128 * (u + 1)], tp[:], AF.Copy)
                                pp = ps_a.tile([128, 512], F32, tag="pp")
                                nc.tensor.matmul(pp[:], h128[:], s1tg[:],
                                                 start=True, stop=True)
                                for u in range(4):
                                    slot = 16 * ch + 4 * sg + u
                                    i = order[slot]
                                    nc.scalar.activation(
                                        pttc[:, 128 * (4 * sg + u):128 * (4 * sg + u + 1)],
                                        pp[:, 128 * u:128 * (u + 1)],
                                        AF.Copy, scale=sign_sc[:, i:i + 1])
                            nc.sync.dma_start(pt_loc[ch][:], pttc[:].bitcast(F32))
                            nc.gpsimd.collective_compute(
                                "AllGather", mybir.AluOpType.bypass,
                                replica_groups=[list(range(NC))],
                                ins=[pt_loc[ch].opt()], outs=[pt_all[ch].opt()])

                # ============ Phase B: up-pass, up-gather, down-pass ============
                def proj_pass(w_in, out_sl_dtype, pool_sfx):
                    with (
                        tc.tile_pool(name=f"pb{pool_sfx}", bufs=8) as pb,
                        tc.tile_pool(name=f"pbw{pool_sfx}", bufs=16) as pbw,
                        tc.tile_pool(name=f"pbo{pool_sfx}", bufs=1) as pbo,
                        tc.tile_pool(name=f"ps_b{pool_sfx}", bufs=1, space="PSUM") as ps_b,
                    ):
                        psu = [ps_b.tile([128, DS], F32, tag=f"psu{m}", name=f"psu{m}")
                               for m in range(NC)]
                        for slot in range(64):
                            ch, il = slot // 16, slot % 16
                            i = order[slot]
                            pti = pb.tile([128, NC * 128], F32R, tag="pti")
                            nc.sync.dma_start(
                                pti[:].rearrange("p (m t) -> p m t", m=NC),
                                pt_all[ch][:, 128 * il:128 * (il + 1)].bitcast(F32R)
                                .rearrange("(m p) t -> p m t", p=128))
                            wi = pbw.tile([128, DS], F32R, tag="wi")
                            nc.sync.dma_start(wi[:], w_in[128 * i:128 * (i + 1), :])
                            for m in range(NC):
                                nc.tensor.matmul(psu[m][:],
                                                 pti[:, 128 * m:128 * (m + 1)], wi[:],
                                                 start=(slot == 0), stop=(slot == 63))
                        sl = pbo.tile([128, NC * DS], out_sl_dtype, name=f"sl{pool_sfx}")
                        for m in range(NC):
                            nc.scalar.activation(sl[:, DS * m:DS * (m + 1)], psu[m][:],
                                                 AF.Copy)
                        return sl

                up_sl = proj_pass(wupt_in, F32R, "u")
                nc.sync.dma_start(
                    up_loc.rearrange("(m p) d -> p m d", p=128),
                    up_sl[:].bitcast(F32).rearrange("p (m d) -> p m d", m=NC))
                nc.gpsimd.collective_compute(
                    "AllGather", mybir.AluOpType.bypass,
                    replica_groups=[list(range(NC))],
                    ins=[up_loc.opt()], outs=[up_all.opt()])

                dn_sl = proj_pass(wdn_in, F32, "d")
                with (
                    tc.tile_pool(name="dnt", bufs=1) as dnt_pool,
                    tc.tile_pool(name="ps_t", bufs=3, space="PSUM") as ps_t,
                ):
                    ident2 = dnt_pool.tile([128, 128], F32)
                    make_identity(nc, ident2[:])
                    dnt = dnt_pool.tile([128, 4 * R], F32R)  # [dp, (a, r)]
                    for a in range(4):
                        for m in range(NC):
                            tp = ps_t.tile([128, 128], F32, tag="tp2")
                            nc.tensor.transpose(
                                tp[:],
                                dn_sl[:, DS * m + 128 * a:DS * m + 128 * (a + 1)],
                                ident2[:])
                            nc.scalar.activation(
                                dnt[:, R * a + 128 * m:R * a + 128 * (m + 1)], tp[:],
                                AF.Copy)
                    nc.sync.dma_start(
                        dn_loc.rearrange("(a p) r -> p a r", p=128),
                        dnt[:].bitcast(F32).rearrange("p (a r) -> p a r", a=4))
                nc.gpsimd.collective_compute(
                    "AllGather", mybir.AluOpType.bypass,
                    replica_groups=[list(range(NC))],
                    ins=[dn_loc.opt()], outs=[dn_all.opt()])

                # ====== mm1: h^T = (x @ up)^T with gelu, streamed to DRAM ======
                with (
                    tc.tile_pool(name="c1s", bufs=3) as c1s,
                    tc.tile_pool(name="ps_c1", bufs=4, space="PSUM") as ps_c1,
                ):
                    for g in range(NC):
                        upg = c1s.tile([128, NC * DS], F32R, tag="upg")  # [p,(rk,d')]
                        nc.sync.dma_start(
                            upg[:].rearrange("p (rk d) -> p rk d", rk=NC),
                            up_all[R * g:R * (g + 1), :].bitcast(F32R)
                            .rearrange("(rk p) d -> p rk d", p=128))
                        for dtg in range(4):
                            for tq in range(4):
                                ph = ps_c1.tile([128, 512], F32, tag="ph")
                                for rk in range(NC):
                                    nc.tensor.matmul(
                                        ph[:],
                                        upg[:, DS * rk + 128 * dtg:DS * rk + 128 * (dtg + 1)],
                                        xt_sb[:, TS * rk + 512 * tq:TS * rk + 512 * (tq + 1)],
                                        start=(rk == 0), stop=(rk == NC - 1))
                                ht = c1s.tile([128, 512], F32R, tag="ht", bufs=4)
                                nc.scalar.activation(ht[:], ph[:], AF.Gelu)
                                d0 = DS * g + 128 * dtg
                                nc.sync.dma_start(
                                    h_dram[d0:d0 + 128, 512 * tq:512 * (tq + 1)]
                                    .bitcast(F32R),
                                    ht[:])

            # ================= mm2: out = h @ down^T =================
            with (
                tc.tile_pool(name="c2", bufs=1) as c2,
                tc.tile_pool(name="c2s", bufs=4) as c2s,
                tc.tile_pool(name="ps_c2", bufs=4, space="PSUM") as ps_c2,
            ):
                dn_a = c2.tile([128, 16 * R], F32R)  # [p, (dk<16, r)]
                nc.sync.dma_start(
                    dn_a[:].rearrange("p (dk r) -> p dk r", dk=16),
                    dn_all[0:2048, :].bitcast(F32R)
                    .rearrange("(dk p) r -> p dk r", p=128))
                dn_b = c2.tile([128, 16 * R], F32R)  # [p, (dk>=16, r)]
                nc.sync.dma_start(
                    dn_b[:].rearrange("p (dk r) -> p dk r", dk=16),
                    dn_all[2048:D, :].bitcast(F32R)
                    .rearrange("(dk p) r -> p dk r", p=128))
                for tt in range(16):
                    hcol = c2s.tile([128, 32 * 128], F32R, tag="hcol")
                    nc.sync.dma_start(
                        hcol[:].rearrange("p (dk t) -> p dk t", dk=32),
                        h_dram[:, 128 * tt:128 * (tt + 1)].bitcast(F32R)
                        .rearrange("(dk p) t -> p dk t", p=128))
                    for rh in range(2):
                        po = ps_c2.tile([128, 512], F32, tag="po")
                        for dk in range(32):
                            src, dkl = (dn_a, dk) if dk < 16 else (dn_b, dk - 16)
                            nc.tensor.matmul(
                                po[:], hcol[:, 128 * dk:128 * (dk + 1)],
                                src[:, R * dkl + 512 * rh:R * dkl + 512 * (rh + 1)],
                                start=(dk == 0), stop=(dk == 31))
                        ot = c2s.tile([128, 512], F32, tag="ot", bufs=4)
                        nc.vector.tensor_copy(ot[:], po[:])
                        nc.sync.dma_start(
                            out_ext[128 * tt:128 * (tt + 1), 512 * rh:512 * (rh + 1)],
                            ot[:])

    nc.compile()
    return nc


def _get_nc():
    global _NC_CACHE
    if _NC_CACHE is None:
        _NC_CACHE = _build()
    return _NC_CACHE


def kernel(x, random_sign, proj_indices, proj_values, w_up, w_down):
    global last_exec_time_ns
    x = np.ascontiguousarray(np.asarray(x, dtype=np.float32))
    sign = np.asarray(random_sign, dtype=np.float32)
    pi = np.asarray(proj_indices)
    pv = np.asarray(proj_values, dtype=np.float32)
    w_up = np.asarray(w_up, dtype=np.float32)
    w_down = np.asarray(w_down, dtype=np.float32)

    # ---- host marshalling ----
    S = np.zeros((R, C), dtype=np.float32)
    np.add.at(S, (pi[0].astype(np.int64), pi[1].astype(np.int64)), pv)
    sign_host = np.ascontiguousarray(sign.reshape(64, 128).T)
    h128 = np.ascontiguousarray(_hadamard(128).astype(np.float32))
    xT = np.ascontiguousarray(x.T)
    wupT = np.ascontiguousarray(w_up.T)

    in_maps = []
    for k in range(NC):
        in_maps.append({
            "s_in": np.ascontiguousarray(S[128 * k:128 * (k + 1), :]),
            "sign_in": sign_host,
            "h128_in": h128,
            "wupt_in": np.ascontiguousarray(wupT[:, DS * k:DS * (k + 1)]),
            "wdn_in": np.ascontiguousarray(w_down[:, DS * k:DS * (k + 1)]),
            "xt_in": np.ascontiguousarray(xT[:, TS * k:TS * (k + 1)]),
        })

    trace = bool(os.environ.get("KERNEL_TRACE"))
    if trace:
        _register_ntff_hook()
    nc = _get_nc()
    res = run_bass_kernel_spmd(nc, in_maps, core_ids=list(range(NC)), trace=trace)
    last_exec_time_ns = res.exec_time_ns
    return np.concatenate([res.results[k]["out"] for k in range(NC)], axis=0)
